# revision 25
# baseline (speedup 1.0000x reference)
import sys

sys.path.insert(0, "/opt/trn_rl_repo")

from contextlib import ExitStack

import numpy as np
import ml_dtypes

import concourse.bass as bass
import concourse.mybir as mybir
import concourse.tile as tile
from concourse import bacc
from concourse.bass_utils import run_bass_kernel_spmd
from concourse.masks import make_identity

H, DIM, DH = 8, 1024, 64
B, N = 2, 2048
NB = N // 128        # 16 row blocks
CC = DIM // 128      # 8 contraction chunks
CH = 256             # channels per core (2 heads x 2*DH)
LAMBDA_INIT = 0.5
RMS_EPS = 1e-5
AF = mybir.ActivationFunctionType
dt = mybir.dt
bf16 = ml_dtypes.bfloat16

# scores-bias add placement: tiles [0, N_DVE) use DVE subs, rest use
# PE identity-matmul accumulate + exp(scale=+-1)
N_DVE = 12
QUADS = [[0, 1, 2, 3], [4, 5, 6, 7], [8, 9, 10, 11]]

_CACHE = {}


def _build():
    nc = bacc.Bacc("TRN2", target_bir_lowering=False, debug=False)
    xT_d = nc.dram_tensor("xT", (DIM, N), dt.bfloat16, kind="ExternalInput").ap()
    wq_d = nc.dram_tensor("wq", (DIM, CH), dt.bfloat16, kind="ExternalInput").ap()
    wk_d = nc.dram_tensor("wk", (DIM, CH), dt.bfloat16, kind="ExternalInput").ap()
    wv_d = nc.dram_tensor("wv", (DIM, CH), dt.bfloat16, kind="ExternalInput").ap()
    wo_d = nc.dram_tensor("wo", (CH, DIM), dt.bfloat16, kind="ExternalInput").ap()
    par_d = nc.dram_tensor("par", (128, 16), dt.float32, kind="ExternalInput").ap()
    out_d = nc.dram_tensor("out", (N, DIM), dt.float32, kind="ExternalOutput").ap()

    with tile.TileContext(nc) as tc, ExitStack() as ctx:
        persist = ctx.enter_context(tc.tile_pool(name="persist", bufs=1))
        par = persist.tile([128, 16], dt.float32)
        nc.sync.dma_start(par, par_d)
        ident = persist.tile([128, 128], dt.bfloat16)
        make_identity(nc, ident)
        # vext: per (row-block, head): 128 V channels + ones col at 128
        vext = persist.tile([128, NB, 2, 132], dt.bfloat16)
        nc.vector.memset(vext[:, :, :, 128:129], 1.0)
        qT = persist.tile([128, 2, N], dt.bfloat16)
        kTD = persist.tile([128, 2, N], dt.bfloat16)   # [k1_hat; -k2_hat] (cols: tokens)
        yT = persist.tile([128, 2, N], dt.bfloat16)
        wo_s = persist.tile([128, 2, DIM], dt.bfloat16)

        # ---- phase 1: projections + l2 normalize (k2 negated via par) ----
        with tc.tile_pool(name="p12", bufs=1) as p12:
            qn = p12.tile([128, NB, CH], dt.bfloat16)
            kn = p12.tile([128, NB, CH], dt.bfloat16)
            wq_s = p12.tile([128, CC, CH], dt.bfloat16)
            wk_s = p12.tile([128, CC, CH], dt.bfloat16)
            wv_s = p12.tile([128, CC, CH], dt.bfloat16)
            with tc.tile_pool(name="xp", bufs=1) as xp, \
                 tc.tile_pool(name="ps1", bufs=2, space="PSUM") as ps1, \
                 tc.tile_pool(name="sb1", bufs=3) as sb1:
                xT_s = xp.tile([128, CC, N], dt.bfloat16)
                xT_r = xT_d.rearrange("(c p) n -> p c n", p=128)
                nc.sync.dma_start(xT_s[:, 0, :], xT_r[:, 0, :])
                nc.sync.dma_start(wq_s, wq_d.rearrange("(c p) h -> p c h", p=128))
                nc.sync.dma_start(wk_s, wk_d.rearrange("(c p) h -> p c h", p=128))
                nc.sync.dma_start(wv_s, wv_d.rearrange("(c p) h -> p c h", p=128))
                for c in range(1, CC):
                    nc.sync.dma_start(xT_s[:, c, :], xT_r[:, c, :])
                nc.sync.dma_start(wo_s, wo_d.rearrange("(c p) o -> p c o", p=128))
                for ib in range(NB):
                    nsl = slice(ib * 128, (ib + 1) * 128)
                    psQ = ps1.tile([128, CH], dt.float32, tag="q")
                    psK = ps1.tile([128, CH], dt.float32, tag="k")
                    psV = ps1.tile([128, CH], dt.float32, tag="v")
                    for c in range(CC):
                        st, sp = (c == 0), (c == CC - 1)
                        nc.tensor.matmul(psQ, lhsT=xT_s[:, c, nsl], rhs=wq_s[:, c, :],
                                         start=st, stop=sp, skip_group_check=True)
                        nc.tensor.matmul(psK, lhsT=xT_s[:, c, nsl], rhs=wk_s[:, c, :],
                                         start=st, stop=sp, skip_group_check=True)
                        nc.tensor.matmul(psV, lhsT=xT_s[:, c, nsl], rhs=wv_s[:, c, :],
                                         start=st, stop=sp, skip_group_check=True)
                    nc.vector.tensor_copy(
                        vext[:, ib, :, 0:128],
                        psV.rearrange("p (h c) -> p h c", h=2))
                    sqq = sb1.tile([128, CH], dt.float32, tag="sqq")
                    sqk = sb1.tile([128, CH], dt.float32, tag="sqk")
                    nc.scalar.activation(sqq, psQ, AF.Square, bias=par[:, 12:13])
                    nc.scalar.activation(sqk, psK, AF.Square, bias=par[:, 12:13])
                    ss = sb1.tile([128, 8], dt.float32, tag="ss")
                    nc.vector.reduce_sum(out=ss[:, 0:4],
                                         in_=sqq.rearrange("p (g d) -> p g d", d=DH),
                                         axis=mybir.AxisListType.X)
                    nc.vector.reduce_sum(out=ss[:, 4:8],
                                         in_=sqk.rearrange("p (g d) -> p g d", d=DH),
                                         axis=mybir.AxisListType.X)
                    nrm = sb1.tile([128, 8], dt.float32, tag="nrm")
                    nc.scalar.activation(nrm, ss, AF.Sqrt, bias=par[:, 12:13])
                    rr = sb1.tile([128, 8], dt.float32, tag="rr")
                    nc.vector.reciprocal(rr, nrm)
                    rs = sb1.tile([128, 8], dt.float32, tag="rs")
                    nc.vector.tensor_mul(rs, rr, par[:, 0:8])
                    for j in range(4):
                        csl = slice(j * DH, (j + 1) * DH)
                        nc.scalar.mul(qn[:, ib, csl], psQ[:, csl], rs[:, j:j + 1])
                        nc.vector.tensor_scalar_mul(kn[:, ib, csl], psK[:, csl], rs[:, 4 + j:5 + j])
            # ---- phase 2: transpose q, k to [d, n] layout ----
            with tc.tile_pool(name="ps2", bufs=4, space="PSUM") as ps2:
                for h in range(2):
                    hs = slice(h * 128, (h + 1) * 128)
                    for ib in range(NB):
                        nsl = slice(ib * 128, (ib + 1) * 128)
                        tq = ps2.tile([128, 128], dt.bfloat16, tag="tq")
                        nc.tensor.transpose(tq, qn[:, ib, hs], ident)
                        nc.scalar.copy(qT[:, h, nsl], tq)
                        tk = ps2.tile([128, 128], dt.bfloat16, tag="tk")
                        nc.tensor.transpose(tk, kn[:, ib, hs], ident)
                        nc.vector.tensor_copy(kTD[:, h, nsl], tk)

        # ---- main loop over 512-query chunks ----
        epool = ctx.enter_context(tc.tile_pool(name="epool", bufs=2))
        sA = ctx.enter_context(tc.tile_pool(name="sA", bufs=4))
        sH = ctx.enter_context(tc.tile_pool(name="sH", bufs=6))
        sC = ctx.enter_context(tc.tile_pool(name="sC", bufs=4))
        sO = ctx.enter_context(tc.tile_pool(name="sO", bufs=3))
        pA = ctx.enter_context(tc.tile_pool(name="pA", bufs=2, space="PSUM"))
        pS = ctx.enter_context(tc.tile_pool(name="pS", bufs=2, space="PSUM"))

        def score_mms(h, n4, im):
            """Emit L12 + D matmuls for one 128-key block; return (L12, D)."""
            msl = slice(im * 128, (im + 1) * 128)
            nsl4 = slice(n4 * 512, (n4 + 1) * 512)
            L12 = pA.tile([128, 1024], dt.float32, tag="L12")
            Dp = pA.tile([128, 512], dt.float32, tag="D")
            nc.tensor.matmul(L12[:, 0:512], lhsT=kTD[0:64, h, msl], rhs=qT[0:64, h, nsl4],
                             start=True, stop=True, skip_group_check=True)
            nc.tensor.matmul(L12[:, 512:1024], lhsT=kTD[64:128, h, msl], rhs=qT[64:128, h, nsl4],
                             start=True, stop=True, skip_group_check=True)
            nc.tensor.matmul(Dp, lhsT=kTD[:, h, msl], rhs=qT[:, h, nsl4],
                             start=True, stop=True, skip_group_check=True)
            return L12, Dp

        for n4 in range(4):
            ms = sC.tile([128, 8], dt.float32, tag="ms")
            zpark = sC.tile([128, 2, 4, 128], dt.bfloat16, tag="zpark")
            es8 = sC.tile([128, 8], dt.float32, tag="es8")
            for h in range(2):
                E = epool.tile([128, NB, 1024], dt.bfloat16, tag="E")
                # DVE-sub tiles (quad-batched exp)
                for quad in QUADS:
                    nq = len(quad)
                    S12 = sA.tile([128, 4, 1024], dt.bfloat16, tag="s12")
                    for j, im in enumerate(quad):
                        L12, Dp = score_mms(h, n4, im)
                        Ht = sH.tile([128, 512], dt.bfloat16, tag="H")
                        # H = tanh(-dg*D) = -G
                        nc.scalar.activation(Ht, Dp, AF.Tanh,
                                             scale=par[:, 8 + h:9 + h], bias=par[:, 12:13])
                        nc.vector.tensor_sub(S12[:, j, 0:512], L12[:, 0:512], Ht)
                        nc.vector.tensor_sub(S12[:, j, 512:1024], Ht, L12[:, 512:1024])
                    nc.scalar.activation(E[:, quad[0]:quad[0] + nq, :], S12[:, 0:nq, :],
                                         AF.Exp, bias=par[:, 12:13])
                # PE-add tiles: software-pipelined so Act never waits in-queue
                prev = None
                for im in range(N_DVE, NB):
                    L12, Dp = score_mms(h, n4, im)
                    Gt = sH.tile([128, 512], dt.bfloat16, tag="H")
                    nc.scalar.activation(Gt, Dp, AF.Tanh,
                                         scale=par[:, 14 + h:15 + h], bias=par[:, 12:13])
                    if prev is not None:
                        _flush_pe_tile(nc, E, par, ident, *prev)
                    prev = (im, L12, Gt)
                if prev is not None:
                    _flush_pe_tile(nc, E, par, ident, *prev)
                # B: attention @ V with free denominators, combine + RMS
                for q in range(4):
                    U1 = pS.tile([128, 512], dt.float32, tag="u")
                    qs1 = slice(q * 128, (q + 1) * 128)
                    qs2 = slice(512 + q * 128, 512 + (q + 1) * 128)
                    for im in range(NB):
                        nc.tensor.matmul(U1[:, 0:129], lhsT=E[:, im, qs1],
                                         rhs=vext[:, im, h, 0:129],
                                         start=(im == 0), stop=(im == NB - 1),
                                         skip_group_check=True)
                    U2 = pS.tile([128, 512], dt.float32, tag="u")
                    for im in range(NB):
                        nc.tensor.matmul(U2[:, 0:129], lhsT=E[:, im, qs2],
                                         rhs=vext[:, im, h, 0:129],
                                         start=(im == 0), stop=(im == NB - 1),
                                         skip_group_check=True)
                    U1c = sC.tile([128, 132], dt.float32, tag="u1c")
                    nc.vector.tensor_copy(U1c[:, 0:129], U1[:, 0:129])
                    r2 = sC.tile([128, 1], dt.float32, tag="r2")
                    nc.vector.reciprocal(r2, U2[:, 128:129])
                    rl = sC.tile([128, 1], dt.float32, tag="rl")
                    nc.vector.tensor_scalar_mul(rl, r2, par[:, 10:11])  # -lam*r2
                    cf = sC.tile([128, 1], dt.float32, tag="cf")
                    nc.vector.tensor_mul(cf, U1c[:, 128:129], rl)      # -lam*s1*r2
                    t2 = sC.tile([128, 128], dt.bfloat16, tag="t2")
                    nc.vector.tensor_scalar_mul(t2, U2[:, 0:128], cf)
                    zp = zpark[:, h, q, :]
                    nc.vector.tensor_add(zp, t2, U1c[:, 0:128])
                    e1 = sC.tile([128, 1], dt.float32, tag="e1")
                    nc.vector.tensor_scalar_mul(e1, U1c[:, 128:129], float(RMS_EPS))
                    nc.vector.tensor_mul(es8[:, 4 * h + q:4 * h + q + 1],
                                         U1c[:, 128:129], e1)
                    ysq = sC.tile([128, 128], dt.float32, tag="ysq")
                    nc.vector.tensor_mul(ysq, zp, zp)
                    nc.vector.reduce_sum(out=ms[:, 4 * h + q:4 * h + q + 1], in_=ysq,
                                         axis=mybir.AxisListType.X)
            # batched RMS scale factors for this chunk (one sqrt -> one table flip)
            msx = sC.tile([128, 8], dt.float32, tag="msx")
            nc.vector.scalar_tensor_tensor(msx, ms, 1.0 / 128.0, es8,
                                           op0=mybir.AluOpType.mult,
                                           op1=mybir.AluOpType.add)
            rho8 = sC.tile([128, 8], dt.float32, tag="rho8")
            nc.scalar.activation(rho8, msx, AF.Sqrt, bias=par[:, 12:13])
            rrho8 = sC.tile([128, 8], dt.float32, tag="rrho8")
            nc.vector.reciprocal(rrho8, rho8)
            for h in range(2):
                for q in range(4):
                    y2 = sC.tile([128, 128], dt.bfloat16, tag="y2")
                    nc.vector.tensor_scalar_mul(y2, zpark[:, h, q, :],
                                                rrho8[:, 4 * h + q:4 * h + q + 1])
                    Tt = pS.tile([128, 512], dt.float32, tag="u")
                    ytr = Tt[:, 0:64].bitcast(dt.bfloat16)
                    nc.tensor.transpose(ytr, y2, ident)
                    nc.vector.tensor_copy(
                        yT[:, h, n4 * 512 + q * 128:n4 * 512 + (q + 1) * 128], ytr)
            for q in range(4):
                nsl = slice(n4 * 512 + q * 128, n4 * 512 + (q + 1) * 128)
                ob = sO.tile([128, 1024], dt.float32, tag="ob")
                for half in range(2):
                    osl = slice(half * 512, (half + 1) * 512)
                    pOt = pS.tile([128, 512], dt.float32, tag="u")
                    for h in range(2):
                        nc.tensor.matmul(pOt, lhsT=yT[:, h, nsl], rhs=wo_s[:, h, osl],
                                         start=(h == 0), stop=(h == 1), skip_group_check=True)
                    nc.vector.tensor_copy(ob[:, osl], pOt)
                nc.sync.dma_start(out_d[nsl, :], ob)

    nc.compile()
    return nc


def _flush_pe_tile(nc, E, par, ident, im, L12, Gt):
    # S1 = L1 + G, S2n = L2n + G accumulated on PE; E2 = exp(-(S2n)) = exp(L2 - G)
    nc.tensor.matmul(L12[:, 0:512], lhsT=ident, rhs=Gt,
                     start=False, stop=True, skip_group_check=True)
    nc.tensor.matmul(L12[:, 512:1024], lhsT=ident, rhs=Gt,
                     start=False, stop=True, skip_group_check=True)
    nc.scalar.activation(E[:, im, 0:512], L12[:, 0:512], AF.Exp,
                         bias=par[:, 12:13], scale=1.0)
    nc.scalar.activation(E[:, im, 512:1024], L12[:, 512:1024], AF.Exp,
                         bias=par[:, 12:13], scale=-1.0)


def kernel(x, Wq, Wk, Wv, Wo, bo,
           lambda_q1, lambda_k1, lambda_q2, lambda_k2,
           delta_gain, cos_head_delta, cos_logit_scale_raw, subln_weight,
           trace=False):
    x = np.asarray(x, np.float32)
    Wq = np.asarray(Wq, np.float32)
    Wk = np.asarray(Wk, np.float32)
    Wv = np.asarray(Wv, np.float32)
    Wo = np.asarray(Wo, np.float32)
    bo = np.asarray(bo, np.float32)

    # host-side scalar prep
    raw = np.float32(cos_logit_scale_raw)
    gscale = 15.0 / (1.0 + np.exp(-raw))
    hd = np.asarray(cos_head_delta, np.float32)
    hd = hd - hd.mean()
    cos_scale = (gscale * (1.0 + 0.5 * np.tanh(hd))).astype(np.float32)  # (H,)
    lam = np.float32(
        np.exp(np.sum(np.asarray(lambda_q1, np.float32) * np.asarray(lambda_k1, np.float32)))
        - np.exp(np.sum(np.asarray(lambda_q2, np.float32) * np.asarray(lambda_k2, np.float32)))
        + LAMBDA_INIT)
    dg = np.asarray(delta_gain, np.float32)
    wsub = (np.asarray(subln_weight, np.float32) * (1.0 - LAMBDA_INIT)).astype(np.float32)
    wsub256 = np.concatenate([wsub, wsub])  # per-channel for a head pair

    if "nc" not in _CACHE:
        _CACHE["nc"] = _build()
    nc = _CACHE["nc"]

    in_maps = []
    for core in range(8):
        b, g = core // 4, core % 4
        h0 = 2 * g
        rows = slice(h0 * 2 * DH, (h0 + 2) * 2 * DH)  # 256 output channels
        par = np.zeros((128, 16), np.float32)
        par[:, 13] = RMS_EPS
        par[:, 0] = cos_scale[h0]
        par[:, 1] = cos_scale[h0]
        par[:, 2] = cos_scale[h0 + 1]
        par[:, 3] = cos_scale[h0 + 1]
        par[:, 4] = 1.0
        par[:, 5] = -1.0   # negate k2 of head h0
        par[:, 6] = 1.0
        par[:, 7] = -1.0   # negate k2 of head h0+1
        par[:, 8] = -dg[h0]       # H-tiles: tanh(-dg*D)
        par[:, 9] = -dg[h0 + 1]
        par[:, 10] = -lam
        par[:, 14] = dg[h0]       # G-tiles: tanh(+dg*D)
        par[:, 15] = dg[h0 + 1]
        # wsub (and the 1-lambda_init factor) folded into wo on host
        wo_host = (Wo[:, rows].T * wsub256[:, None]).astype(bf16)
        in_maps.append({
            "xT": np.ascontiguousarray(x[b].T).astype(bf16),
            "wq": np.ascontiguousarray(Wq[rows].T).astype(bf16),
            "wk": np.ascontiguousarray(Wk[rows].T).astype(bf16),
            "wv": np.ascontiguousarray(Wv[rows].T).astype(bf16),
            "wo": np.ascontiguousarray(wo_host),
            "par": par,
        })

    res = run_bass_kernel_spmd(nc, in_maps, core_ids=list(range(8)), trace=trace)
    outs = [res.results[c]["out"] for c in range(8)]
    full = np.zeros((B, N, DIM), np.float32)
    for b in range(B):
        acc = outs[4 * b].astype(np.float32)
        for g in range(1, 4):
            acc = acc + outs[4 * b + g].astype(np.float32)
        full[b] = acc + bo[None, :]
    if trace:
        return full, res
    return full


# revision 26
# speedup vs baseline: 1.2141x; 1.2141x over previous
import sys

sys.path.insert(0, "/opt/trn_rl_repo")

from contextlib import ExitStack

import numpy as np
import ml_dtypes

import concourse.bass as bass
import concourse.mybir as mybir
import concourse.tile as tile
from concourse import bacc
from concourse.bass_utils import run_bass_kernel_spmd
from concourse.masks import make_identity

H, DIM, DH = 8, 1024, 64
B, N = 2, 2048
NB = N // 128        # 16 row blocks
CC = DIM // 128      # 8 contraction chunks
CH = 256             # channels per core (2 heads x 2*DH)
LAMBDA_INIT = 0.5
RMS_EPS = 1e-5
AF = mybir.ActivationFunctionType
dt = mybir.dt
bf16 = ml_dtypes.bfloat16

# scores-bias add placement: tiles [0, N_DVE) use DVE subs, rest use
# PE identity-matmul accumulate + exp(scale=+-1)
N_DVE = 11
QUADS = [[0, 1, 2, 3], [4, 5, 6, 7], [8, 9, 10]]

_CACHE = {}


def _build():
    nc = bacc.Bacc("TRN2", target_bir_lowering=False, debug=False)
    xT_d = nc.dram_tensor("xT", (DIM, N), dt.bfloat16, kind="ExternalInput").ap()
    wq_d = nc.dram_tensor("wq", (DIM, CH), dt.bfloat16, kind="ExternalInput").ap()
    wk_d = nc.dram_tensor("wk", (DIM, CH), dt.bfloat16, kind="ExternalInput").ap()
    wv_d = nc.dram_tensor("wv", (DIM, CH), dt.bfloat16, kind="ExternalInput").ap()
    wo_d = nc.dram_tensor("wo", (CH, DIM), dt.bfloat16, kind="ExternalInput").ap()
    par_d = nc.dram_tensor("par", (128, 16), dt.float32, kind="ExternalInput").ap()
    out_d = nc.dram_tensor("out", (N, DIM), dt.float32, kind="ExternalOutput").ap()

    with tile.TileContext(nc) as tc, ExitStack() as ctx:
        persist = ctx.enter_context(tc.tile_pool(name="persist", bufs=1))
        par = persist.tile([128, 16], dt.float32)
        nc.sync.dma_start(par, par_d)
        ident = persist.tile([128, 128], dt.bfloat16)
        make_identity(nc, ident)
        # vext: per (row-block, head): 128 V channels + ones col at 128
        vext = persist.tile([128, NB, 2, 132], dt.bfloat16)
        nc.vector.memset(vext[:, :, :, 128:129], 1.0)
        qT = persist.tile([128, 2, N], dt.bfloat16)
        kTD = persist.tile([128, 2, N], dt.bfloat16)   # [k1_hat; -k2_hat] (cols: tokens)
        yT = persist.tile([128, 2, N], dt.bfloat16)
        wo_s = persist.tile([128, 2, DIM], dt.bfloat16)

        # ---- phase 1: projections + l2 normalize (k2 negated via par) ----
        with tc.tile_pool(name="p12", bufs=1) as p12:
            qn = p12.tile([128, NB, CH], dt.bfloat16)
            kn = p12.tile([128, NB, CH], dt.bfloat16)
            wq_s = p12.tile([128, CC, CH], dt.bfloat16)
            wk_s = p12.tile([128, CC, CH], dt.bfloat16)
            wv_s = p12.tile([128, CC, CH], dt.bfloat16)
            with tc.tile_pool(name="xp", bufs=1) as xp, \
                 tc.tile_pool(name="ps1", bufs=2, space="PSUM") as ps1, \
                 tc.tile_pool(name="sb1", bufs=3) as sb1:
                xT_s = xp.tile([128, CC, N], dt.bfloat16)
                xT_r = xT_d.rearrange("(c p) n -> p c n", p=128)
                nc.sync.dma_start(xT_s[:, 0, :], xT_r[:, 0, :])
                nc.sync.dma_start(wq_s, wq_d.rearrange("(c p) h -> p c h", p=128))
                nc.sync.dma_start(wk_s, wk_d.rearrange("(c p) h -> p c h", p=128))
                nc.sync.dma_start(wv_s, wv_d.rearrange("(c p) h -> p c h", p=128))
                for c in range(1, CC):
                    nc.sync.dma_start(xT_s[:, c, :], xT_r[:, c, :])
                nc.sync.dma_start(wo_s, wo_d.rearrange("(c p) o -> p c o", p=128))
                for ib in range(NB):
                    nsl = slice(ib * 128, (ib + 1) * 128)
                    psQ = ps1.tile([128, CH], dt.float32, tag="q")
                    psK = ps1.tile([128, CH], dt.float32, tag="k")
                    psV = ps1.tile([128, CH], dt.float32, tag="v")
                    for c in range(CC):
                        st, sp = (c == 0), (c == CC - 1)
                        nc.tensor.matmul(psQ, lhsT=xT_s[:, c, nsl], rhs=wq_s[:, c, :],
                                         start=st, stop=sp, skip_group_check=True)
                        nc.tensor.matmul(psK, lhsT=xT_s[:, c, nsl], rhs=wk_s[:, c, :],
                                         start=st, stop=sp, skip_group_check=True)
                        nc.tensor.matmul(psV, lhsT=xT_s[:, c, nsl], rhs=wv_s[:, c, :],
                                         start=st, stop=sp, skip_group_check=True)
                    nc.vector.tensor_copy(
                        vext[:, ib, :, 0:128],
                        psV.rearrange("p (h c) -> p h c", h=2))
                    sqq = sb1.tile([128, CH], dt.float32, tag="sqq")
                    sqk = sb1.tile([128, CH], dt.float32, tag="sqk")
                    nc.scalar.activation(sqq, psQ, AF.Square, bias=par[:, 12:13])
                    nc.scalar.activation(sqk, psK, AF.Square, bias=par[:, 12:13])
                    ss = sb1.tile([128, 8], dt.float32, tag="ss")
                    nc.vector.reduce_sum(out=ss[:, 0:4],
                                         in_=sqq.rearrange("p (g d) -> p g d", d=DH),
                                         axis=mybir.AxisListType.X)
                    nc.vector.reduce_sum(out=ss[:, 4:8],
                                         in_=sqk.rearrange("p (g d) -> p g d", d=DH),
                                         axis=mybir.AxisListType.X)
                    nrm = sb1.tile([128, 8], dt.float32, tag="nrm")
                    nc.scalar.activation(nrm, ss, AF.Sqrt, bias=par[:, 12:13])
                    rr = sb1.tile([128, 8], dt.float32, tag="rr")
                    nc.vector.reciprocal(rr, nrm)
                    rs = sb1.tile([128, 8], dt.float32, tag="rs")
                    nc.vector.tensor_mul(rs, rr, par[:, 0:8])
                    for j in range(4):
                        csl = slice(j * DH, (j + 1) * DH)
                        nc.scalar.mul(qn[:, ib, csl], psQ[:, csl], rs[:, j:j + 1])
                        nc.vector.tensor_scalar_mul(kn[:, ib, csl], psK[:, csl], rs[:, 4 + j:5 + j])
            # ---- phase 2: transpose q, k to [d, n] layout ----
            with tc.tile_pool(name="ps2", bufs=4, space="PSUM") as ps2:
                for h in range(2):
                    hs = slice(h * 128, (h + 1) * 128)
                    for ib in range(NB):
                        nsl = slice(ib * 128, (ib + 1) * 128)
                        tq = ps2.tile([128, 128], dt.bfloat16, tag="tq")
                        nc.tensor.transpose(tq, qn[:, ib, hs], ident)
                        nc.scalar.copy(qT[:, h, nsl], tq)
                        tk = ps2.tile([128, 128], dt.bfloat16, tag="tk")
                        nc.tensor.transpose(tk, kn[:, ib, hs], ident)
                        nc.vector.tensor_copy(kTD[:, h, nsl], tk)

        # ---- main loop over 512-query chunks ----
        epool = ctx.enter_context(tc.tile_pool(name="epool", bufs=2))
        sA = ctx.enter_context(tc.tile_pool(name="sA", bufs=4))
        sH = ctx.enter_context(tc.tile_pool(name="sH", bufs=6))
        sC = ctx.enter_context(tc.tile_pool(name="sC", bufs=4))
        sO = ctx.enter_context(tc.tile_pool(name="sO", bufs=3))
        pA = ctx.enter_context(tc.tile_pool(name="pA", bufs=2, space="PSUM"))
        pS = ctx.enter_context(tc.tile_pool(name="pS", bufs=2, space="PSUM"))

        def score_mms(h, n4, im):
            """Emit L12 + D matmuls for one 128-key block; return (L12, D)."""
            msl = slice(im * 128, (im + 1) * 128)
            nsl4 = slice(n4 * 512, (n4 + 1) * 512)
            L12 = pA.tile([128, 1024], dt.float32, tag="L12")
            Dp = pA.tile([128, 512], dt.float32, tag="D")
            nc.tensor.matmul(L12[:, 0:512], lhsT=kTD[0:64, h, msl], rhs=qT[0:64, h, nsl4],
                             start=True, stop=True, skip_group_check=True)
            nc.tensor.matmul(L12[:, 512:1024], lhsT=kTD[64:128, h, msl], rhs=qT[64:128, h, nsl4],
                             start=True, stop=True, skip_group_check=True)
            nc.tensor.matmul(Dp, lhsT=kTD[:, h, msl], rhs=qT[:, h, nsl4],
                             start=True, stop=True, skip_group_check=True)
            return L12, Dp

        for n4 in range(4):
            ms = sC.tile([128, 8], dt.float32, tag="ms")
            zpark = sC.tile([128, 2, 4, 128], dt.bfloat16, tag="zpark")
            es8 = sC.tile([128, 8], dt.float32, tag="es8")
            for h in range(2):
                E = epool.tile([128, NB, 1024], dt.bfloat16, tag="E")
                # DVE-sub tiles (quad-batched exp)
                for quad in QUADS:
                    nq = len(quad)
                    S12 = sA.tile([128, 4, 1024], dt.bfloat16, tag="s12")
                    for j, im in enumerate(quad):
                        L12, Dp = score_mms(h, n4, im)
                        Ht = sH.tile([128, 512], dt.bfloat16, tag="H")
                        # H = tanh(-dg*D) = -G
                        nc.scalar.activation(Ht, Dp, AF.Tanh,
                                             scale=par[:, 8 + h:9 + h], bias=par[:, 12:13])
                        nc.vector.tensor_sub(S12[:, j, 0:512], L12[:, 0:512], Ht)
                        nc.vector.tensor_sub(S12[:, j, 512:1024], Ht, L12[:, 512:1024])
                    nc.scalar.activation(E[:, quad[0]:quad[0] + nq, :], S12[:, 0:nq, :],
                                         AF.Exp, bias=par[:, 12:13])
                # PE-add tiles: software-pipelined so Act never waits in-queue
                prev = None
                for im in range(N_DVE, NB):
                    L12, Dp = score_mms(h, n4, im)
                    Gt = sH.tile([128, 512], dt.bfloat16, tag="H")
                    nc.scalar.activation(Gt, Dp, AF.Tanh,
                                         scale=par[:, 14 + h:15 + h], bias=par[:, 12:13])
                    if prev is not None:
                        _flush_pe_tile(nc, E, par, ident, *prev)
                    prev = (im, L12, Gt)
                if prev is not None:
                    _flush_pe_tile(nc, E, par, ident, *prev)
                # B: attention @ V with free denominators, combine + RMS
                for q in range(4):
                    U1 = pS.tile([128, 512], dt.float32, tag="u")
                    qs1 = slice(q * 128, (q + 1) * 128)
                    qs2 = slice(512 + q * 128, 512 + (q + 1) * 128)
                    for im in range(NB):
                        nc.tensor.matmul(U1[:, 0:129], lhsT=E[:, im, qs1],
                                         rhs=vext[:, im, h, 0:129],
                                         start=(im == 0), stop=(im == NB - 1),
                                         skip_group_check=True)
                    U2 = pS.tile([128, 512], dt.float32, tag="u")
                    for im in range(NB):
                        nc.tensor.matmul(U2[:, 0:129], lhsT=E[:, im, qs2],
                                         rhs=vext[:, im, h, 0:129],
                                         start=(im == 0), stop=(im == NB - 1),
                                         skip_group_check=True)
                    U1c = sC.tile([128, 132], dt.float32, tag="u1c")
                    nc.vector.tensor_copy(U1c[:, 0:129], U1[:, 0:129])
                    r2 = sC.tile([128, 1], dt.float32, tag="r2")
                    nc.vector.reciprocal(r2, U2[:, 128:129])
                    rl = sC.tile([128, 1], dt.float32, tag="rl")
                    nc.vector.tensor_scalar_mul(rl, r2, par[:, 10:11])  # -lam*r2
                    cf = sC.tile([128, 1], dt.float32, tag="cf")
                    nc.vector.tensor_mul(cf, U1c[:, 128:129], rl)      # -lam*s1*r2
                    t2 = sC.tile([128, 128], dt.bfloat16, tag="t2")
                    nc.vector.tensor_scalar_mul(t2, U2[:, 0:128], cf)
                    zp = zpark[:, h, q, :]
                    nc.vector.tensor_add(zp, t2, U1c[:, 0:128])
                    e1 = sC.tile([128, 1], dt.float32, tag="e1")
                    nc.vector.tensor_scalar_mul(e1, U1c[:, 128:129], float(RMS_EPS))
                    nc.vector.tensor_mul(es8[:, 4 * h + q:4 * h + q + 1],
                                         U1c[:, 128:129], e1)
                    ysq = sC.tile([128, 128], dt.float32, tag="ysq")
                    nc.vector.tensor_mul(ysq, zp, zp)
                    nc.vector.reduce_sum(out=ms[:, 4 * h + q:4 * h + q + 1], in_=ysq,
                                         axis=mybir.AxisListType.X)
            # batched RMS scale factors for this chunk (one sqrt -> one table flip)
            msx = sC.tile([128, 8], dt.float32, tag="msx")
            nc.vector.scalar_tensor_tensor(msx, ms, 1.0 / 128.0, es8,
                                           op0=mybir.AluOpType.mult,
                                           op1=mybir.AluOpType.add)
            rho8 = sC.tile([128, 8], dt.float32, tag="rho8")
            nc.scalar.activation(rho8, msx, AF.Sqrt, bias=par[:, 12:13])
            rrho8 = sC.tile([128, 8], dt.float32, tag="rrho8")
            nc.vector.reciprocal(rrho8, rho8)
            for h in range(2):
                for q in range(4):
                    y2 = sC.tile([128, 128], dt.bfloat16, tag="y2")
                    nc.vector.tensor_scalar_mul(y2, zpark[:, h, q, :],
                                                rrho8[:, 4 * h + q:4 * h + q + 1])
                    Tt = pS.tile([128, 512], dt.float32, tag="u")
                    ytr = Tt[:, 0:64].bitcast(dt.bfloat16)
                    nc.tensor.transpose(ytr, y2, ident)
                    nc.vector.tensor_copy(
                        yT[:, h, n4 * 512 + q * 128:n4 * 512 + (q + 1) * 128], ytr)
            for q in range(4):
                nsl = slice(n4 * 512 + q * 128, n4 * 512 + (q + 1) * 128)
                ob = sO.tile([128, 1024], dt.float32, tag="ob")
                for half in range(2):
                    osl = slice(half * 512, (half + 1) * 512)
                    pOt = pS.tile([128, 512], dt.float32, tag="u")
                    for h in range(2):
                        nc.tensor.matmul(pOt, lhsT=yT[:, h, nsl], rhs=wo_s[:, h, osl],
                                         start=(h == 0), stop=(h == 1), skip_group_check=True)
                    nc.vector.tensor_copy(ob[:, osl], pOt)
                nc.sync.dma_start(out_d[nsl, :], ob)

    nc.compile()
    return nc


def _flush_pe_tile(nc, E, par, ident, im, L12, Gt):
    # S1 = L1 + G, S2n = L2n + G accumulated on PE; E2 = exp(-(S2n)) = exp(L2 - G)
    nc.tensor.matmul(L12[:, 0:512], lhsT=ident, rhs=Gt,
                     start=False, stop=True, skip_group_check=True)
    nc.tensor.matmul(L12[:, 512:1024], lhsT=ident, rhs=Gt,
                     start=False, stop=True, skip_group_check=True)
    nc.scalar.activation(E[:, im, 0:512], L12[:, 0:512], AF.Exp,
                         bias=par[:, 12:13], scale=1.0)
    nc.scalar.activation(E[:, im, 512:1024], L12[:, 512:1024], AF.Exp,
                         bias=par[:, 12:13], scale=-1.0)


def kernel(x, Wq, Wk, Wv, Wo, bo,
           lambda_q1, lambda_k1, lambda_q2, lambda_k2,
           delta_gain, cos_head_delta, cos_logit_scale_raw, subln_weight,
           trace=False):
    x = np.asarray(x, np.float32)
    Wq = np.asarray(Wq, np.float32)
    Wk = np.asarray(Wk, np.float32)
    Wv = np.asarray(Wv, np.float32)
    Wo = np.asarray(Wo, np.float32)
    bo = np.asarray(bo, np.float32)

    # host-side scalar prep
    raw = np.float32(cos_logit_scale_raw)
    gscale = 15.0 / (1.0 + np.exp(-raw))
    hd = np.asarray(cos_head_delta, np.float32)
    hd = hd - hd.mean()
    cos_scale = (gscale * (1.0 + 0.5 * np.tanh(hd))).astype(np.float32)  # (H,)
    lam = np.float32(
        np.exp(np.sum(np.asarray(lambda_q1, np.float32) * np.asarray(lambda_k1, np.float32)))
        - np.exp(np.sum(np.asarray(lambda_q2, np.float32) * np.asarray(lambda_k2, np.float32)))
        + LAMBDA_INIT)
    dg = np.asarray(delta_gain, np.float32)
    wsub = (np.asarray(subln_weight, np.float32) * (1.0 - LAMBDA_INIT)).astype(np.float32)
    wsub256 = np.concatenate([wsub, wsub])  # per-channel for a head pair

    if "nc" not in _CACHE:
        _CACHE["nc"] = _build()
    nc = _CACHE["nc"]

    in_maps = []
    for core in range(8):
        b, g = core // 4, core % 4
        h0 = 2 * g
        rows = slice(h0 * 2 * DH, (h0 + 2) * 2 * DH)  # 256 output channels
        par = np.zeros((128, 16), np.float32)
        par[:, 13] = RMS_EPS
        par[:, 0] = cos_scale[h0]
        par[:, 1] = cos_scale[h0]
        par[:, 2] = cos_scale[h0 + 1]
        par[:, 3] = cos_scale[h0 + 1]
        par[:, 4] = 1.0
        par[:, 5] = -1.0   # negate k2 of head h0
        par[:, 6] = 1.0
        par[:, 7] = -1.0   # negate k2 of head h0+1
        par[:, 8] = -dg[h0]       # H-tiles: tanh(-dg*D)
        par[:, 9] = -dg[h0 + 1]
        par[:, 10] = -lam
        par[:, 14] = dg[h0]       # G-tiles: tanh(+dg*D)
        par[:, 15] = dg[h0 + 1]
        # wsub (and the 1-lambda_init factor) folded into wo on host
        wo_host = (Wo[:, rows].T * wsub256[:, None]).astype(bf16)
        in_maps.append({
            "xT": np.ascontiguousarray(x[b].T).astype(bf16),
            "wq": np.ascontiguousarray(Wq[rows].T).astype(bf16),
            "wk": np.ascontiguousarray(Wk[rows].T).astype(bf16),
            "wv": np.ascontiguousarray(Wv[rows].T).astype(bf16),
            "wo": np.ascontiguousarray(wo_host),
            "par": par,
        })

    res = run_bass_kernel_spmd(nc, in_maps, core_ids=list(range(8)), trace=trace)
    outs = [res.results[c]["out"] for c in range(8)]
    full = np.zeros((B, N, DIM), np.float32)
    for b in range(B):
        acc = outs[4 * b].astype(np.float32)
        for g in range(1, 4):
            acc = acc + outs[4 * b + g].astype(np.float32)
        full[b] = acc + bo[None, :]
    if trace:
        return full, res
    return full
